# revision 2
# baseline (speedup 1.0000x reference)
"""Trainium2 Bass kernel for nn_Decoder (mapping MLP + hard-LSTM scan + out proj).

Self-contained: takes FULL inputs (as produced by setup_inputs), shards batch
across 8 NeuronCores, runs a Bass/Tile kernel via run_bass_kernel_spmd, and
gathers the full [T, K, B, C] output.

Layout per core (B' = B/8 = 512 batch elems):
  rows = k*B' + b  (20 "rtiles" of 512 rows each, one per k)
  h, c state: [H=128 partitions, 512 rows] bf16 tiles, one pair per rtile.

Per step & rtile (engine assignment tuned against the TRN2 cost model):
  PE   : 4 W_ih-[x;1] preloads + 4 W_hh matmuls fill two PSUM bank-pairs
         [i|g] and [f|o]; the out-projection runs TRANSPOSED (h 128-batch
         chunk as stationary, W_outT as moving) so it costs ~2 columns
         instead of 512, accumulating [128 batch, 2] slots in a PSUM bank.
  ACT  : one merged relu evac of [f|o] -> a_fo bf16; PSUM out-bank evac
         every 64 rtile-steps.
  DVE  : OP_T3 t = hs(i)*ht(g) straight from both PSUM banks;
         OP_UC u = min(a_f,1)*c; OP_H h = min(a_o,1)*ht(c) (lagged 1 rtile
         so the GPSIMD add can complete without stalling the DVE stream).
  POOL : c = u + t (native tensor add on the otherwise idle GPSIMD engine).
"""
import os
import sys

sys.path.insert(0, "/opt/trn_rl_repo")

import numpy as np
import ml_dtypes
from contextlib import ExitStack

import concourse.bass as bass
import concourse.tile as tile
from concourse import mybir, bacc
import concourse.dve_ops as _dve_ops_mod
from concourse.dve_ops import DveOp, OPS, CUSTOM_DVE_SPECS, _CUSTOM_DVE_ROW_BASE
from concourse.dve_spec import (
    Spec, Src0, Src1, C0, C1, C2, Zero, One, maxx, minn, relu, lower, _has_src1,
)
from concourse.dve_uop import DveOpSpec
from concourse.bass_utils import run_bass_kernel_spmd

FP32 = mybir.dt.float32
BF16 = mybir.dt.bfloat16
AF = mybir.ActivationFunctionType

# Full-problem config (hardcoded; the harness always calls with these shapes).
T_FULL, K_FULL, B_FULL, C_DIM, H_DIM, MH_DIM, N_CORES = 20, 20, 4096, 2, 128, 64, 8


# ---------------------------------------------------------------- custom ops
def _register_op(name, spec):
    for op in OPS:
        if op.name == name:
            return op
    shas = {}
    for ver in ("v3", "v4"):
        tmp = DveOpSpec(name=name, opcode=0, uops=lower(spec, ver=ver),
                        rd1_en=_has_src1(spec))
        shas[ver] = tmp.sha(ver)
    op = DveOp(name, spec, subdim=False, uops_sha=shas)
    OPS.append(op)
    CUSTOM_DVE_SPECS[name] = spec
    _dve_ops_mod._SUB_OPCODE_FOR_NAME[name] = _CUSTOM_DVE_ROW_BASE + len(OPS) - 1
    assert _dve_ops_mod._SUB_OPCODE_FOR_NAME[name] < 0x20
    return op


def _hs(x, s0, s1):
    return np.minimum(np.maximum(x * s0 + s1, 0.0), 1.0)


def _ht(x, lo):
    return np.maximum(np.minimum(x, 1.0), lo)


# t = hs(i)*ht(g), i prescaled ((raw+b)/6+0.5) in psum, g raw+b in psum.
OP_T3 = _register_op(
    "ANT_LSTM_T3",
    Spec(body=minn(relu(Src0 * C0 + C1), One) * maxx(minn(Src1, One), C2),
         reference=lambda in0, in1, s0, s1, imm2:
             _hs(in0, s0, s1) * _ht(in1, imm2)),
)
# u = min(a_f, 1) * c ; a_f = relu(f') from ACT
OP_UC = _register_op(
    "ANT_LSTM_UC",
    Spec(body=minn(Src0, One) * Src1,
         reference=lambda in0, in1, s0, s1, imm2: np.minimum(in0, 1.0) * in1),
)
# h = min(a_o,1)*clip(c, -1, 1); a_o = relu(o') from ACT; imm2=-1
OP_H = _register_op(
    "ANT_LSTM_H",
    Spec(body=minn(Src0, One) * maxx(minn(Src1, One), C2),
         reference=lambda in0, in1, s0, s1, imm2:
             np.minimum(in0, 1.0) * np.maximum(np.minimum(in1, 1.0), imm2)),
)
# leaky_relu(psum + bm1) = max(y, 0.01*y), y = Src0 + C1; imm2 = slope
OP_LRELU = _register_op(
    "ANT_LRELU",
    Spec(body=maxx(Src0 + C1, (Src0 + C1) * C2),
         reference=lambda in0, in1, s0, s1, imm2:
             np.maximum(in0 + s1, (in0 + s1) * imm2)),
)


# ---------------------------------------------------------------- bass build
def build_nc(T, K, BP, use_pack=True, repeat=1, tmp_bufs=4,
             c_add_eng="gpsimd", out_lag=2, **_unused):
    """Build the per-core Bass program. BP = per-core batch (must be 512).

    PSUM banks (8 x 512 fp32): pa = [i | g] x2 bufs (DVE-read), pb = [f | o]
    x1 buf (ACT-evac'd early), po = out-projection slot banks x2 bufs.
    Biases + hardsigmoid pre-scale folded into the augmented W_ih-[x;1]
    preload on the host: i/f/o psum arrive as (raw+b)/6+0.5, g as raw+b.
    """
    H, MH, C = H_DIM, MH_DIM, C_DIM
    CA = C + 1        # augmented x rows (x0, x1, 1)
    FD = BP           # free dim of every tile
    PSB = 512         # PSUM bank stride in fp32 elems
    RT = K            # rtiles per step
    assert FD == PSB, "layout assumes BP == 512"
    nc = bacc.Bacc("TRN2", target_bir_lowering=False, debug=False)

    phT_e = nc.declare_dram_parameter("phT", [H, K * BP], BF16, isOutput=False)
    xh_e = nc.declare_dram_parameter("xh", [CA, T * BP], BF16, isOutput=False)
    whhT_e = nc.declare_dram_parameter("whhT", [H, 4 * H], BF16, isOutput=False)
    wihT_e = nc.declare_dram_parameter("wihT", [CA, 4 * H], BF16, isOutput=False)
    woutT_e = nc.declare_dram_parameter("woutT", [H, C], BF16, isOutput=False)
    wm1T_e = nc.declare_dram_parameter("wm1T", [H, MH], BF16, isOutput=False)
    wm2T_e = nc.declare_dram_parameter("wm2T", [MH, H], BF16, isOutput=False)
    biasp_e = nc.declare_dram_parameter("biasp", [H, 2], FP32, isOutput=False)
    # out: [128 batch-chunk partitions, T*K*4chunks*2coords] fp32
    NOUT = T * K * 4 * C
    out_e = nc.declare_dram_parameter("out", [128, NOUT], FP32, isOutput=True)

    with tile.TileContext(nc) as tc:
        with ExitStack() as ctx:
            wts = ctx.enter_context(tc.tile_pool(name="wts", bufs=1))
            big = ctx.enter_context(tc.tile_pool(name="big", bufs=1))
            st = ctx.enter_context(tc.tile_pool(name="st", bufs=1))
            tmp = ctx.enter_context(tc.tile_pool(name="tmp", bufs=tmp_bufs))
            psA = ctx.enter_context(tc.tile_pool(name="psA", bufs=2, space="PSUM"))
            psB = ctx.enter_context(tc.tile_pool(name="psB", bufs=1, space="PSUM"))
            psO = ctx.enter_context(tc.tile_pool(name="psO", bufs=1, space="PSUM"))

            # ---- weights / constants into SBUF
            whhT = wts.tile([H, 4 * H], BF16, tag="whhT")
            nc.sync.dma_start(whhT[:], whhT_e[:])
            woutT = wts.tile([H, C], BF16, tag="woutT")
            nc.sync.dma_start(woutT[:], woutT_e[:])
            wm1T = wts.tile([H, MH], BF16, tag="wm1T")
            nc.sync.dma_start(wm1T[:], wm1T_e[:])
            wm2T = wts.tile([MH, H], BF16, tag="wm2T")
            nc.sync.dma_start(wm2T[:], wm2T_e[:])
            biasp = wts.tile([H, 2], FP32, tag="biasp")
            nc.sync.dma_start(biasp[:], biasp_e[:])

            npack = 4 if use_pack else 1
            nprows = 32 * (npack - 1) + CA
            wih = wts.tile([nprows, 4 * H], BF16, tag="wih")
            xrep = wts.tile([nprows, T * BP], BF16, tag="xrep")
            for r in range(npack):
                nc.sync.dma_start(wih[32 * r:32 * r + CA, :], wihT_e[:])
                nc.sync.dma_start(xrep[32 * r:32 * r + CA, :], xh_e[:])

            phT = big.tile([H, K * BP], BF16, tag="phT")
            nc.sync.dma_start(phT[:], phT_e[:])

            # SBUF accumulation buffer for the final output
            out_sb = big.tile([128, NOUT], FP32, tag="out_sb")

            # persistent out-projection psum slot banks (2 bufs, 1 bank each)
            po_t = [psO.tile([128, PSB], FP32, name=f"po{q}", tag=f"po{q}")
                    for q in range(2)]

            # ---- persistent state tiles
            h_t = [st.tile([H, FD], BF16, name=f"h{j}", tag=f"h{j}")
                   for j in range(RT)]
            c_t = [st.tile([H, FD], BF16, name=f"c{j}", tag=f"c{j}")
                   for j in range(RT)]

            # ---- mapping MLP -> h0
            for j in range(RT):
                pa = psA.tile([H, 2 * PSB], FP32, tag="pa")
                nc.tensor.matmul(pa[0:MH, 0:FD], wm1T[:, 0:MH],
                                 phT[:, j * FD:(j + 1) * FD],
                                 start=True, stop=True)
                a1 = tmp.tile([MH, FD], BF16, tag="a1")
                nc.vector._custom_dve(OP_LRELU, out=a1[:], in0=pa[0:MH, 0:FD],
                                      s1=biasp[0:MH, 0:1], imm2=0.01)
                nc.tensor.matmul(pa[0:H, PSB:PSB + FD], wm2T[:, 0:H], a1[:],
                                 start=True, stop=True)
                nc.scalar.activation(h_t[j][:], pa[0:H, PSB:PSB + FD],
                                     AF.Identity, bias=biasp[:, 1:2], scale=1.0)

            # gate chunk offsets in whhT / wih cols: i=0, f=1, g=2, o=3
            CH = {"i": 0, "f": 1, "g": 2, "o": 3}

            def gcol(name):
                m = CH[name]
                return slice(m * H, (m + 1) * H)

            def gates_mm(ps, xcols, j, names):
                # W_ih-[x;1] preloads (contract=3, row-packed), then W_hh accum
                for r, gname in enumerate(names):
                    rr = (CH[gname] if use_pack else 0)
                    sl = slice(0, FD) if r == 0 else slice(PSB, PSB + FD)
                    nc.tensor.matmul(
                        ps[:, sl],
                        wih[32 * rr:32 * rr + CA, gcol(gname)],
                        xrep[32 * rr:32 * rr + CA, xcols],
                        start=True, stop=False,
                        tile_position=(32 * rr, 0) if use_pack else None,
                    )
                for r, gname in enumerate(names):
                    sl = slice(0, FD) if r == 0 else slice(PSB, PSB + FD)
                    nc.tensor.matmul(ps[:, sl], whhT[:, gcol(gname)],
                                     h_t[j][:], start=False, stop=True)

            # pending work queues for software pipelining
            pend_h = []     # (a_fo_tile, j) -> emit OP_H
            pend_out = []   # (so, j) -> emit 4 transposed out-proj matmuls
            evac_cnt = [0]  # po evacs so far (picks po buffer)

            def emit_h(a_fo, j):
                nc.vector._custom_dve(OP_H, out=h_t[j][:],
                                      in0=a_fo[:, PSB:PSB + FD],
                                      in1=c_t[j][:], imm2=-1.0)

            def emit_out(so, j):
                sm = so % (T * RT)
                po = po_t[(sm // 64) % 2]
                col = (sm % 64) * (4 * C)
                for q in range(4):
                    nc.tensor.matmul(po[:, col + C * q:col + C * (q + 1)],
                                     h_t[j][:, 128 * q:128 * (q + 1)],
                                     woutT[:, 0:C], start=True, stop=True)
                if sm % 64 == 63 or sm == T * RT - 1:
                    n = (sm % 64 + 1) * (4 * C)
                    base = (sm - sm % 64) * (4 * C)
                    nc.scalar.activation(out_sb[:, base:base + n],
                                         po[:, 0:n], AF.Copy,
                                         bias=0.0, scale=1.0)
                    evac_cnt[0] += 1

            # ---- time loop (repeat>1 is for timing only)
            for tl in range(T * repeat):
                t = tl % T
                xcols = slice(t * BP, (t + 1) * BP)
                for j in range(RT):
                    pb = psB.tile([H, 2 * PSB], FP32, tag="pb")  # [f | o]
                    pa = psA.tile([H, 2 * PSB], FP32, tag="pa")  # [i | g]
                    gates_mm(pb, xcols, j, "fo")
                    # ACT: merged a_fo = relu([f|o] psum) (scale/bias prefolded)
                    a_fo = tmp.tile([H, 2 * PSB], BF16, tag="afo")
                    nc.scalar.activation(a_fo[:], pb[:, 0:2 * PSB], AF.Relu,
                                         bias=0.0, scale=1.0)
                    gates_mm(pa, xcols, j, "ig")

                    # DVE: t = hs(i)*ht(g) straight from the two psum banks
                    if t == 0:
                        t_dst = c_t[j]      # c0 = 0 -> c1 = t
                    else:
                        t_dst = tmp.tile([H, FD], BF16, tag="tt")
                    nc.vector._custom_dve(OP_T3, out=t_dst[:],
                                          in0=pa[:, 0:FD],
                                          in1=pa[:, PSB:PSB + FD],
                                          s0=1.0, s1=0.0, imm2=-1.0)
                    if t > 0:
                        u_d = tmp.tile([H, FD], BF16, tag="uu")
                        nc.vector._custom_dve(OP_UC, out=u_d[:],
                                              in0=a_fo[:, 0:FD],
                                              in1=c_t[j][:])
                    # lagged OP_H for the previous rtile
                    if pend_h:
                        emit_h(*pend_h.pop(0))
                    if t > 0:
                        add_e = getattr(nc, c_add_eng)
                        add_e.tensor_add(c_t[j][:], u_d[:], t_dst[:])
                    pend_h.append((a_fo, j))

                    # lagged transposed out-projection
                    pend_out.append((tl * RT + j, j))
                    if len(pend_out) > out_lag:
                        emit_out(*pend_out.pop(0))
                # end of step: flush the lagged OP_H (deps are ready)
                while pend_h:
                    emit_h(*pend_h.pop(0))
            while pend_out:
                emit_out(*pend_out.pop(0))

            nc.sync.dma_start(out_e[:], out_sb[:])

    nc.finalize()
    return nc


# ---------------------------------------------------------------- host side
def _bf16(x):
    return np.ascontiguousarray(x, dtype=np.float32).astype(ml_dtypes.bfloat16)


def prep_core_inputs(inputs, core, T, K, BP):
    H, MH, C = H_DIM, MH_DIM, C_DIM
    b0 = core * BP
    ph = np.asarray(inputs["pred_lstm_hidden"], np.float32)[:, b0:b0 + BP, :]
    phT = ph.transpose(2, 0, 1).reshape(H, K * BP)
    idx = np.concatenate([[0], np.arange(T - 1)])
    obs = np.asarray(inputs["obs_traj_rel"], np.float32)
    xs = obs[idx][:, b0:b0 + BP, :C]
    xh = xs.transpose(2, 0, 1).reshape(C, T * BP)
    xh = np.concatenate([xh, np.ones((1, T * BP), np.float32)], axis=0)
    bsum = (np.asarray(inputs["b_ih"], np.float32)
            + np.asarray(inputs["b_hh"], np.float32))
    # per-gate-chunk scale and bias folded into W_hh / W_ih / the x=1 row:
    #   i, f, o chunks: psum = (raw + b)/6 + 0.5 ; g chunk: psum = raw + b
    scale = np.ones(4 * H, np.float32) / 6.0
    scale[2 * H:3 * H] = 1.0
    bias_row = bsum * scale
    bias_row[0:2 * H] += 0.5
    bias_row[3 * H:4 * H] += 0.5
    whh_s = np.asarray(inputs["W_hh"], np.float32) * scale[:, None]
    wih_s = np.asarray(inputs["W_ih"], np.float32) * scale[:, None]
    wih_aug = np.concatenate([wih_s.T, bias_row[None, :]], axis=0)  # [3, 4H]
    biasp = np.zeros((H, 2), np.float32)
    biasp[0:MH, 0] = np.asarray(inputs["bm1"], np.float32)
    biasp[:, 1] = np.asarray(inputs["bm2"], np.float32)
    return {
        "phT": _bf16(phT),
        "xh": _bf16(xh),
        "whhT": _bf16(whh_s.T),
        "wihT": _bf16(wih_aug),
        "woutT": _bf16(np.asarray(inputs["W_out"], np.float32).T),
        "wm1T": _bf16(np.asarray(inputs["Wm1"], np.float32).T),
        "wm2T": _bf16(np.asarray(inputs["Wm2"], np.float32).T),
        "biasp": biasp,
    }


_NC_CACHE = {}


def _get_nc(T, K, BP):
    key = (T, K, BP)
    if key not in _NC_CACHE:
        _NC_CACHE[key] = build_nc(T, K, BP)
    return _NC_CACHE[key]


def kernel(**inputs) -> np.ndarray:
    T, K, B, C = T_FULL, K_FULL, B_FULL, C_DIM
    BP = B // N_CORES
    nc = _get_nc(T, K, BP)
    in_maps = [prep_core_inputs(inputs, c, T, K, BP) for c in range(N_CORES)]
    trace = bool(int(os.environ.get("KERNEL_TRACE", "0")))
    res = run_bass_kernel_spmd(nc, in_maps, list(range(N_CORES)), trace=trace)
    if trace:
        kernel.last_exec_time_ns = res.exec_time_ns
        kernel.last_results = res
    # per-core out: [128, T*K*4*C] -> [T, K, BP, C]
    parts = []
    for c in range(N_CORES):
        arr = res.results[c]["out"].reshape(128, T, K, 4, C)
        parts.append(arr.transpose(1, 2, 3, 0, 4).reshape(T, K, BP, C))
    full = np.concatenate(parts, axis=2)  # [T, K, B, C]
    b_out = np.asarray(inputs["b_out"], np.float32)
    return np.ascontiguousarray(full + b_out, dtype=np.float32)
